# revision 3
# baseline (speedup 1.0000x reference)
"""Trainium2 Bass kernel for multiresolution hash-grid encoding (Instant-NGP style).

Contract: kernel(x01, tables) -> [N, 16] float32, computed on 8 NeuronCores.

Strategy:
  Host (weight-layout preprocessing, independent of x01):
    For each level l, expand the hash table into a dense per-cell "cube grid":
    row c (cell id) holds the 8 corner embeddings (8 corners x 2 feats = 16
    f32 = 64B contiguous). Valid because the hash index of every dense grid
    cell is a constant: idx = (ix*P1 ^ iy*P2 ^ iz*P3) mod 2^18, and masking
    distributes over XOR.
  Device (data-parallel over points, SPMD on 8 cores):
    Per 128-point group: DVE computes pos = x*R, frac, integer cell id (exact
    f32 math, cell < 2^24), casts to int32; one indirect DMA gathers the 128
    64B cube rows (one per partition); DVE does the trilinear interpolation as
    3 lerp stages on strided APs. Output written level-major [8, N, 2] and
    reassembled on host.
"""
import math
import numpy as np

NUM_LEVELS = 8
FEATS = 2
TABLE_SIZE = 2 ** 18
MIN_RES = 16
MAX_RES = 256
GROWTH = math.exp(math.log(MAX_RES / MIN_RES) / (NUM_LEVELS - 1))
P1, P2, P3 = 1540863, 1256879, 1957123
RES = [int(math.floor(MIN_RES * GROWTH ** l + 1e-6)) for l in range(NUM_LEVELS)]

N_CORES = 8
P = 128          # SBUF partitions
TP = 2048        # points per partition per core
T_B = 64         # points per partition per batch
PTS_PER_CORE = P * TP          # 262144
N_PAD = N_CORES * PTS_PER_CORE  # 2097152


def _build_cube_grids(tables: np.ndarray) -> list[np.ndarray]:
    """Per level: [R^3, 16] f32 rows; row = 8 corner embeddings of the cell."""
    grids = []
    mask = TABLE_SIZE - 1
    for l, R in enumerate(RES):
        n = R + 1
        vx = (np.arange(n, dtype=np.int64) * P1)
        vy = (np.arange(n, dtype=np.int64) * P2)
        vz = (np.arange(n, dtype=np.int64) * P3)
        corner_idx = (vx[:, None, None] ^ vy[None, :, None] ^ vz[None, None, :]) & mask
        corner_emb = tables[l][corner_idx]  # [n, n, n, 2] f32
        cube = np.empty((R, R, R, 8, FEATS), dtype=np.float32)
        e = 0
        for dx in (0, 1):
            for dy in (0, 1):
                for dz in (0, 1):
                    cube[:, :, :, e, :] = corner_emb[dx:dx + R, dy:dy + R, dz:dz + R]
                    e += 1
        grids.append(np.ascontiguousarray(cube.reshape(R * R * R, 8 * FEATS)))
    return grids


def _build_program():
    import concourse.bass as bass
    import concourse.bacc as bacc
    import concourse.tile as tile
    from concourse import mybir

    f32 = mybir.dt.float32
    i32 = mybir.dt.int32
    Alu = mybir.AluOpType

    nc = bacc.Bacc("TRN2", target_bir_lowering=False, debug=False)
    x_ext = nc.dram_tensor("x", [P, TP, 3], f32, kind="ExternalInput")
    g_ext = [
        nc.dram_tensor(f"g{l}", [RES[l] ** 3, 16], f32, kind="ExternalInput")
        for l in range(NUM_LEVELS)
    ]
    out_ext = nc.dram_tensor("out", [NUM_LEVELS, P, TP, FEATS], f32,
                             kind="ExternalOutput")

    n_batches = TP // T_B
    T = T_B

    with tile.TileContext(nc) as tc:
        with (
            tc.tile_pool(name="xp", bufs=2) as xp,
            tc.tile_pool(name="coord", bufs=3) as coord,
            tc.tile_pool(name="idxp", bufs=3) as idxp,
            tc.tile_pool(name="cubep", bufs=3) as cubep,
            tc.tile_pool(name="lerpp", bufs=3) as lerpp,
            tc.tile_pool(name="outp", bufs=3) as outp,
        ):
            for b in range(n_batches):
                xt = xp.tile([P, T * 3], f32, tag="x")
                nc.sync.dma_start(out=xt[:], in_=x_ext.ap()[:, b * T:(b + 1) * T, :])
                x3 = xt[:].rearrange("p (t c) -> p t c", c=3)
                for l in range(NUM_LEVELS):
                    R = RES[l]
                    pos = coord.tile([P, T * 3], f32, tag="pos")
                    nc.vector.tensor_scalar_mul(pos[:], xt[:], float(R))
                    # floor(pos), robust to cast rounding mode (HW rounds to
                    # nearest): i0f = cast(pos); i0f -= (i0f > pos)
                    icst = coord.tile([P, T * 3], i32, tag="icst")
                    nc.vector.tensor_copy(out=icst[:], in_=pos[:])
                    i0f = coord.tile([P, T * 3], f32, tag="i0f")
                    nc.vector.tensor_copy(out=i0f[:], in_=icst[:])
                    up = coord.tile([P, T * 3], f32, tag="up")
                    nc.vector.tensor_tensor(
                        out=up[:], in0=i0f[:], in1=pos[:], op=Alu.is_gt
                    )
                    nc.vector.tensor_tensor(
                        out=i0f[:], in0=i0f[:], in1=up[:], op=Alu.subtract
                    )
                    frac = coord.tile([P, T * 3], f32, tag="frac")
                    nc.vector.tensor_tensor(
                        out=frac[:], in0=pos[:], in1=i0f[:], op=Alu.subtract
                    )
                    # clamp to [0, R-1] (guard only; x in [0,1) never hits it)
                    nc.vector.tensor_scalar_min(i0f[:], i0f[:], float(R - 1))

                    i3 = i0f[:].rearrange("p (t c) -> p t c", c=3)
                    ix, iy, iz = i3[:, :, 0], i3[:, :, 1], i3[:, :, 2]
                    cell = coord.tile([P, T], f32, tag="cell")
                    nc.vector.scalar_tensor_tensor(
                        out=cell[:], in0=ix, scalar=float(R), in1=iy,
                        op0=Alu.mult, op1=Alu.add,
                    )
                    nc.vector.scalar_tensor_tensor(
                        out=cell[:], in0=cell[:], scalar=float(R), in1=iz,
                        op0=Alu.mult, op1=Alu.add,
                    )
                    idx = idxp.tile([P, T], i32, tag="idx")
                    nc.vector.tensor_copy(out=idx[:], in_=cell[:])

                    cube = cubep.tile([P, T * 16], f32, tag="cube")
                    for j in range(T):
                        nc.gpsimd.indirect_dma_start(
                            out=cube[:, j * 16:(j + 1) * 16],
                            out_offset=None,
                            in_=g_ext[l].ap(),
                            in_offset=bass.IndirectOffsetOnAxis(
                                ap=idx[:, j:j + 1], axis=0
                            ),
                        )

                    f3 = frac[:].rearrange("p (t c) -> p t c", c=3)
                    fx, fy, fz = f3[:, :, 0], f3[:, :, 1], f3[:, :, 2]

                    # z lerp: [T,4,2] = E0 + fz*(E1-E0); cube flat idx
                    # = 8dx+4dy+2dz+f
                    cz = cube[:].rearrange("p (t a z f) -> p t a z f", a=4, z=2, f=2)
                    e0, e1 = cz[:, :, :, 0, :], cz[:, :, :, 1, :]
                    az = lerpp.tile([P, T * 8], f32, tag="az")
                    az4 = az[:].rearrange("p (t a f) -> p t a f", a=4, f=2)
                    dz = lerpp.tile([P, T * 8], f32, tag="dz")
                    dz4 = dz[:].rearrange("p (t a f) -> p t a f", a=4, f=2)
                    nc.vector.tensor_tensor(out=dz4, in0=e1, in1=e0, op=Alu.subtract)
                    nc.vector.tensor_tensor(
                        out=dz4, in0=dz4, in1=fz.to_broadcast([P, T, 4, 2]),
                        op=Alu.mult,
                    )
                    nc.vector.tensor_tensor(out=az4, in0=dz4, in1=e0, op=Alu.add)

                    # y lerp: [T,2,2]
                    ay = lerpp.tile([P, T * 4], f32, tag="ay")
                    ay4 = ay[:].rearrange("p (t a f) -> p t a f", a=2, f=2)
                    azy = az[:].rearrange("p (t a y f) -> p t a y f", a=2, y=2, f=2)
                    y0, y1 = azy[:, :, :, 0, :], azy[:, :, :, 1, :]
                    dy_ = lerpp.tile([P, T * 4], f32, tag="dy")
                    dy4 = dy_[:].rearrange("p (t a f) -> p t a f", a=2, f=2)
                    nc.vector.tensor_tensor(out=dy4, in0=y1, in1=y0, op=Alu.subtract)
                    nc.vector.tensor_tensor(
                        out=dy4, in0=dy4, in1=fy.to_broadcast([P, T, 2, 2]),
                        op=Alu.mult,
                    )
                    nc.vector.tensor_tensor(out=ay4, in0=dy4, in1=y0, op=Alu.add)

                    # x lerp: [T,2]
                    ol = outp.tile([P, T * 2], f32, tag="ol")
                    ol2 = ol[:].rearrange("p (t f) -> p t f", f=2)
                    ayx = ay[:].rearrange("p (t x f) -> p t x f", x=2, f=2)
                    x0, x1 = ayx[:, :, 0, :], ayx[:, :, 1, :]
                    dx_ = lerpp.tile([P, T * 2], f32, tag="dx")
                    dx2 = dx_[:].rearrange("p (t f) -> p t f", f=2)
                    nc.vector.tensor_tensor(out=dx2, in0=x1, in1=x0, op=Alu.subtract)
                    nc.vector.tensor_tensor(
                        out=dx2, in0=dx2, in1=fx.to_broadcast([P, T, 2]),
                        op=Alu.mult,
                    )
                    nc.vector.tensor_tensor(out=ol2, in0=dx2, in1=x0, op=Alu.add)

                    nc.sync.dma_start(
                        out=out_ext.ap()[l, :, b * T:(b + 1) * T, :],
                        in_=ol[:],
                    )
    nc.compile()
    return nc


_PROGRAM_CACHE = {}


def kernel(x01: np.ndarray, tables: np.ndarray, _trace: bool = False,
           _tmpdir: str | None = None) -> np.ndarray:
    from concourse.bass_utils import run_bass_kernel_spmd

    N = x01.shape[0]
    assert N <= N_PAD, (N, N_PAD)

    grids = _build_cube_grids(np.asarray(tables, dtype=np.float32))

    xp = np.zeros((N_PAD, 3), dtype=np.float32)
    xp[:N] = np.asarray(x01, dtype=np.float32)

    key = "prog"
    if key not in _PROGRAM_CACHE:
        _PROGRAM_CACHE[key] = _build_program()
    nc = _PROGRAM_CACHE[key]

    in_maps = []
    for c in range(N_CORES):
        m = {"x": xp[c * PTS_PER_CORE:(c + 1) * PTS_PER_CORE].reshape(P, TP, 3)}
        for l in range(NUM_LEVELS):
            m[f"g{l}"] = grids[l]
        in_maps.append(m)

    res = run_bass_kernel_spmd(
        nc, in_maps, core_ids=list(range(N_CORES)),
        trace=_trace, tmpdir=_tmpdir,
    )

    # out per core: [8, P, TP, 2] -> global [8, N_PAD, 2] -> [N, 16]
    parts = [r["out"].reshape(NUM_LEVELS, PTS_PER_CORE, FEATS) for r in res.results]
    full = np.concatenate(parts, axis=1)          # [8, N_PAD, 2]
    out = full.transpose(1, 0, 2).reshape(N_PAD, NUM_LEVELS * FEATS)
    if _trace:
        kernel.last_exec_time_ns = res.exec_time_ns
        kernel.last_results = res
    return np.ascontiguousarray(out[:N])


# revision 9
# speedup vs baseline: 1.3278x; 1.3278x over previous
"""Trainium2 Bass kernel for multiresolution hash-grid encoding (Instant-NGP style).

Contract: kernel(x01, tables) -> [N, 16] float32, computed on 8 NeuronCores.

Strategy:
  Host (weight-layout preprocessing, independent of x01):
    For each level l, expand the hash table into a dense per-cell "cube grid":
    row c (cell id) holds the 8 corner embeddings (8 corners x 2 feats = 16
    f32 = 64B contiguous). Valid because the hash index of every dense grid
    cell is a constant: idx = (ix*P1 ^ iy*P2 ^ iz*P3) mod 2^18, and masking
    distributes over XOR.
  Device (data-parallel over points, SPMD on 8 cores):
    Per 128-point group: DVE computes pos = x*R, frac, integer cell id (exact
    f32 math, cell < 2^24), casts to int32; one indirect DMA gathers the 128
    64B cube rows (one per partition); DVE does the trilinear interpolation as
    3 lerp stages on strided APs. Output written level-major [8, N, 2] and
    reassembled on host.
"""
import math
import numpy as np

NUM_LEVELS = 8
FEATS = 2
TABLE_SIZE = 2 ** 18
MIN_RES = 16
MAX_RES = 256
GROWTH = math.exp(math.log(MAX_RES / MIN_RES) / (NUM_LEVELS - 1))
P1, P2, P3 = 1540863, 1256879, 1957123
RES = [int(math.floor(MIN_RES * GROWTH ** l + 1e-6)) for l in range(NUM_LEVELS)]

N_CORES = 8
P = 128          # SBUF partitions
TP = 2048        # points per partition per core
T_B = 64         # points per partition per batch
PTS_PER_CORE = P * TP          # 262144
N_PAD = N_CORES * PTS_PER_CORE  # 2097152


def _build_cube_grids(tables: np.ndarray) -> list[np.ndarray]:
    """Per level: [R^3, 16] f32 rows; row = 8 corner embeddings of the cell."""
    grids = []
    mask = TABLE_SIZE - 1
    for l, R in enumerate(RES):
        n = R + 1
        vx = (np.arange(n, dtype=np.int64) * P1)
        vy = (np.arange(n, dtype=np.int64) * P2)
        vz = (np.arange(n, dtype=np.int64) * P3)
        corner_idx = (vx[:, None, None] ^ vy[None, :, None] ^ vz[None, None, :]) & mask
        corner_emb = tables[l][corner_idx]  # [n, n, n, 2] f32
        cube = np.empty((R, R, R, 8, FEATS), dtype=np.float32)
        e = 0
        for dx in (0, 1):
            for dy in (0, 1):
                for dz in (0, 1):
                    cube[:, :, :, e, :] = corner_emb[dx:dx + R, dy:dy + R, dz:dz + R]
                    e += 1
        grids.append(np.ascontiguousarray(cube.reshape(R * R * R, 8 * FEATS)))
    # Co-locate the lvl0 cube in the lvl7 row: 16 | 256, so the lvl0 cell is
    # cell7 coords >> 4. Row becomes 32 floats: [lvl7 cube | lvl0 cube].
    R7 = RES[7]
    s = R7 // RES[0]
    c0 = grids[0].reshape(RES[0], RES[0], RES[0], 16)
    c0e = np.repeat(np.repeat(np.repeat(c0, s, axis=0), s, axis=1), s, axis=2)
    grids[7] = np.ascontiguousarray(
        np.concatenate([grids[7].reshape(R7, R7, R7, 16), c0e], axis=3)
        .reshape(R7 ** 3, 32)
    )
    return grids


def _build_program():
    import concourse.bass as bass
    import concourse.bacc as bacc
    import concourse.tile as tile
    from concourse import mybir

    f32 = mybir.dt.float32
    i32 = mybir.dt.int32
    Alu = mybir.AluOpType

    nc = bacc.Bacc("TRN2", target_bir_lowering=False, debug=False)
    x_ext = nc.dram_tensor("x", [P, TP, 3], f32, kind="ExternalInput")
    g_ext = {
        l: nc.dram_tensor(f"g{l}", [RES[l] ** 3, 16], f32, kind="ExternalInput")
        for l in range(1, NUM_LEVELS - 1)
    }
    g_ext[7] = nc.dram_tensor("g7", [RES[7] ** 3, 32], f32, kind="ExternalInput")
    out_ext = nc.dram_tensor("out", [NUM_LEVELS, P, TP, FEATS], f32,
                             kind="ExternalOutput")

    n_batches = TP // T_B
    T = T_B

    with tile.TileContext(nc) as tc:
        with (
            tc.tile_pool(name="xp", bufs=2) as xp,
            tc.tile_pool(name="coord", bufs=5) as coord,
            tc.tile_pool(name="idxp", bufs=5) as idxp,
            tc.tile_pool(name="cubep", bufs=4) as cubep,
            tc.tile_pool(name="lerpp", bufs=4) as lerpp,
            tc.tile_pool(name="outp", bufs=4) as outp,
        ):
            for b in range(n_batches):
                xt = xp.tile([P, T * 3], f32, tag="x")
                nc.sync.dma_start(out=xt[:], in_=x_ext.ap()[:, b * T:(b + 1) * T, :])
                x3 = xt[:].rearrange("p (t c) -> p t c", c=3)
                cube7_tile = None
                for l in (7, 0, 1, 2, 3, 4, 5, 6):
                    R = RES[l]
                    pos = coord.tile([P, T * 3], f32, tag="pos")
                    nc.vector.tensor_scalar_mul(pos[:], xt[:], float(R))
                    # floor(pos), robust to cast rounding mode (HW rounds to
                    # nearest): i0f = cast(pos); i0f -= (i0f > pos)
                    icst = coord.tile([P, T * 3], i32, tag="icst")
                    nc.vector.tensor_copy(out=icst[:], in_=pos[:])
                    i0f = coord.tile([P, T * 3], f32, tag="i0f")
                    nc.vector.tensor_copy(out=i0f[:], in_=icst[:])
                    up = coord.tile([P, T * 3], f32, tag="up")
                    nc.vector.tensor_tensor(
                        out=up[:], in0=i0f[:], in1=pos[:], op=Alu.is_gt
                    )
                    nc.vector.tensor_tensor(
                        out=i0f[:], in0=i0f[:], in1=up[:], op=Alu.subtract
                    )
                    frac = coord.tile([P, T * 3], f32, tag="frac")
                    nc.vector.tensor_tensor(
                        out=frac[:], in0=pos[:], in1=i0f[:], op=Alu.subtract
                    )
                    # clamp to [0, R-1] (guard only; x in [0,1) never hits it)
                    nc.vector.tensor_scalar_min(i0f[:], i0f[:], float(R - 1))

                    if l != 0:
                        i3 = i0f[:].rearrange("p (t c) -> p t c", c=3)
                        ix, iy, iz = i3[:, :, 0], i3[:, :, 1], i3[:, :, 2]
                        cell = coord.tile([P, T], f32, tag="cell")
                        nc.vector.scalar_tensor_tensor(
                            out=cell[:], in0=ix, scalar=float(R), in1=iy,
                            op0=Alu.mult, op1=Alu.add,
                        )
                        nc.vector.scalar_tensor_tensor(
                            out=cell[:], in0=cell[:], scalar=float(R), in1=iz,
                            op0=Alu.mult, op1=Alu.add,
                        )
                        idx = idxp.tile([P, T], i32, tag="idx")
                        nc.vector.tensor_copy(out=idx[:], in_=cell[:])

                    rowf = 32 if l == 7 else 16
                    if l == 7:
                        cube = cubep.tile([P, T * 32], f32, tag="cube7")
                        cube7_tile = cube
                    elif l == 0:
                        cube = cube7_tile  # lvl0 cube rides in the lvl7 row
                    else:
                        cube = cubep.tile([P, T * 16], f32, tag="cube")
                    if l != 0:
                        for j in range(T):
                            nc.gpsimd.indirect_dma_start(
                                out=cube[:, j * rowf:(j + 1) * rowf],
                                out_offset=None,
                                in_=g_ext[l].ap(),
                                in_offset=bass.IndirectOffsetOnAxis(
                                    ap=idx[:, j:j + 1], axis=0
                                ),
                            )

                    f3 = frac[:].rearrange("p (t c) -> p t c", c=3)
                    fx, fy, fz = f3[:, :, 0], f3[:, :, 1], f3[:, :, 2]

                    # z lerp: [T,4,2] = E0 + fz*(E1-E0); cube flat idx
                    # = 8dx+4dy+2dz+f (within its 16-float half)
                    if l in (7, 0):
                        c6 = cube[:].rearrange(
                            "p (t h a z f) -> p t h a z f", h=2, a=4, z=2, f=2
                        )
                        h = 0 if l == 7 else 1
                        e0, e1 = c6[:, :, h, :, 0, :], c6[:, :, h, :, 1, :]
                    else:
                        cz = cube[:].rearrange(
                            "p (t a z f) -> p t a z f", a=4, z=2, f=2
                        )
                        e0, e1 = cz[:, :, :, 0, :], cz[:, :, :, 1, :]
                    az = lerpp.tile([P, T * 8], f32, tag="az")
                    az4 = az[:].rearrange("p (t a f) -> p t a f", a=4, f=2)
                    dz = lerpp.tile([P, T * 8], f32, tag="dz")
                    dz4 = dz[:].rearrange("p (t a f) -> p t a f", a=4, f=2)
                    nc.vector.tensor_tensor(out=dz4, in0=e1, in1=e0, op=Alu.subtract)
                    nc.vector.tensor_tensor(
                        out=dz4, in0=dz4, in1=fz.to_broadcast([P, T, 4, 2]),
                        op=Alu.mult,
                    )
                    nc.vector.tensor_tensor(out=az4, in0=dz4, in1=e0, op=Alu.add)

                    # y lerp: [T,2,2]
                    ay = lerpp.tile([P, T * 4], f32, tag="ay")
                    ay4 = ay[:].rearrange("p (t a f) -> p t a f", a=2, f=2)
                    azy = az[:].rearrange("p (t a y f) -> p t a y f", a=2, y=2, f=2)
                    y0, y1 = azy[:, :, :, 0, :], azy[:, :, :, 1, :]
                    dy_ = lerpp.tile([P, T * 4], f32, tag="dy")
                    dy4 = dy_[:].rearrange("p (t a f) -> p t a f", a=2, f=2)
                    nc.vector.tensor_tensor(out=dy4, in0=y1, in1=y0, op=Alu.subtract)
                    nc.vector.tensor_tensor(
                        out=dy4, in0=dy4, in1=fy.to_broadcast([P, T, 2, 2]),
                        op=Alu.mult,
                    )
                    nc.vector.tensor_tensor(out=ay4, in0=dy4, in1=y0, op=Alu.add)

                    # x lerp: [T,2]
                    ol = outp.tile([P, T * 2], f32, tag="ol")
                    ol2 = ol[:].rearrange("p (t f) -> p t f", f=2)
                    ayx = ay[:].rearrange("p (t x f) -> p t x f", x=2, f=2)
                    x0, x1 = ayx[:, :, 0, :], ayx[:, :, 1, :]
                    dx_ = lerpp.tile([P, T * 2], f32, tag="dx")
                    dx2 = dx_[:].rearrange("p (t f) -> p t f", f=2)
                    nc.vector.tensor_tensor(out=dx2, in0=x1, in1=x0, op=Alu.subtract)
                    nc.vector.tensor_tensor(
                        out=dx2, in0=dx2, in1=fx.to_broadcast([P, T, 2]),
                        op=Alu.mult,
                    )
                    nc.vector.tensor_tensor(out=ol2, in0=dx2, in1=x0, op=Alu.add)

                    nc.sync.dma_start(
                        out=out_ext.ap()[l, :, b * T:(b + 1) * T, :],
                        in_=ol[:],
                    )
    nc.compile()
    return nc


_PROGRAM_CACHE = {}


def kernel(x01: np.ndarray, tables: np.ndarray, _trace: bool = False,
           _tmpdir: str | None = None) -> np.ndarray:
    from concourse.bass_utils import run_bass_kernel_spmd

    N = x01.shape[0]
    assert N <= N_PAD, (N, N_PAD)

    grids = _build_cube_grids(np.asarray(tables, dtype=np.float32))

    xp = np.zeros((N_PAD, 3), dtype=np.float32)
    xp[:N] = np.asarray(x01, dtype=np.float32)

    key = "prog"
    if key not in _PROGRAM_CACHE:
        _PROGRAM_CACHE[key] = _build_program()
    nc = _PROGRAM_CACHE[key]

    in_maps = []
    for c in range(N_CORES):
        m = {"x": xp[c * PTS_PER_CORE:(c + 1) * PTS_PER_CORE].reshape(P, TP, 3)}
        for l in range(1, NUM_LEVELS):
            m[f"g{l}"] = grids[l]
        in_maps.append(m)

    res = run_bass_kernel_spmd(
        nc, in_maps, core_ids=list(range(N_CORES)),
        trace=_trace, tmpdir=_tmpdir,
    )

    # out per core: [8, P, TP, 2] -> global [8, N_PAD, 2] -> [N, 16]
    parts = [r["out"].reshape(NUM_LEVELS, PTS_PER_CORE, FEATS) for r in res.results]
    full = np.concatenate(parts, axis=1)          # [8, N_PAD, 2]
    out = full.transpose(1, 0, 2).reshape(N_PAD, NUM_LEVELS * FEATS)
    if _trace:
        kernel.last_exec_time_ns = res.exec_time_ns
        kernel.last_results = res
    return np.ascontiguousarray(out[:N])
